# revision 42
# baseline (speedup 1.0000x reference)
"""MultiHeadAttention forward on 8 Trainium2 NeuronCores.

Problem: x[2,2048,1024] -> fused QKV proj -> 16-head attention -> out proj.
Sharding: (batch=2) x (head-groups=4) across 8 cores. Core c handles
batch b=c//4 and heads 4g..4g+3 where g=c%4.  Per core:
  - QKV projection for its 4 heads (feature-major for q,k; token-major for v)
  - scoresT[k,q] = K^T-major scores, exp on ScalarE (scale=1/8 fused,
    no max-subtraction: scores are bounded ~|8| for this distribution)
  - PV matmul with ones-augmented V -> softmax denominators for free
  - normalize on DVE, out-projection against the head-rows of W_out
Host: slice/permutate/cast inputs, then sum the 4 head-group partial
outputs per batch (the row-parallel all-reduce equivalent).
"""

import math
import numpy as np
import ml_dtypes

import concourse.bass as bass
import concourse.bacc as bacc
import concourse.tile as tile
from concourse import mybir
from concourse.alu_op_type import AluOpType
from concourse.bass_utils import run_bass_kernel_spmd

BF16 = ml_dtypes.bfloat16

B, S, E = 2, 2048, 1024
H, D = 16, 64
HG = 4              # heads per core
N_CORES = 8
P = 128

F32 = mybir.dt.float32
F32R = mybir.dt.float32r
BF = mybir.dt.bfloat16
EXP = mybir.ActivationFunctionType.Exp

_COMPILED = None  # (nc,) cache


def build_program():
    nc = bacc.Bacc("TRN2", target_bir_lowering=False, debug=False)

    xT_d = nc.dram_tensor("xT", [E, S], BF, kind="ExternalInput").ap()
    wqk02_d = nc.dram_tensor("wqk02", [E, 2 * P], BF, kind="ExternalInput").ap()
    wqk13_d = nc.dram_tensor("wqk13", [E, 2 * P], BF, kind="ExternalInput").ap()
    wv_d = nc.dram_tensor("wv", [E, HG * D], BF, kind="ExternalInput").ap()
    wout_d = nc.dram_tensor("wout", [HG * D, E], BF, kind="ExternalInput").ap()
    bqk_d = nc.dram_tensor("bqk", [P, 4], F32, kind="ExternalInput").ap()
    bv_d = nc.dram_tensor("bv", [1, HG * D], F32, kind="ExternalInput").ap()
    bout_d = nc.dram_tensor("bout", [1, E], F32, kind="ExternalInput").ap()
    out_d = nc.dram_tensor("out", [S, E], F32, kind="ExternalOutput").ap()

    ET = E // P   # 8 e-tiles
    ST = S // P   # 16 s-tiles

    with tile.TileContext(nc) as tc:
        with (
            tc.tile_pool(name="consts", bufs=1) as consts,
            tc.tile_pool(name="xin", bufs=9) as xin,
            tc.tile_pool(name="qkt", bufs=1) as qkt_pool,
            tc.tile_pool(name="vaug", bufs=1) as vaug_pool,
            tc.tile_pool(name="expp", bufs=20) as expp,
            tc.tile_pool(name="attn", bufs=1) as attnp,
            tc.tile_pool(name="outsb", bufs=3) as outsb,
            tc.tile_pool(name="rbp", bufs=6) as rbp,
            tc.tile_pool(name="psS", bufs=2, space="PSUM") as psS,
            tc.tile_pool(name="psW", bufs=4, space="PSUM") as psW,
        ):
            # ---- constants / weights (batched DMAs, spread over queues) ----
            qs = [nc.gpsimd, nc.sync]
            wqk02 = consts.tile([P, ET, 2 * P], BF, tag="wqk02", name="wqk02")
            nc.gpsimd.dma_start(
                wqk02, wqk02_d.rearrange("(e p) c -> p e c", p=P))

            # persistent activations
            # qkT m-tiles: 0=q(h0,h1) 1=q(h2,h3) 2=k(h0,h1) 3=k(h2,h3);
            # within a tile partitions 0:64 = even head, 64:128 = odd head.
            qkT = [[qkt_pool.tile([P, 512], BF, tag=f"qkT{m}_{s4}",
                                  name=f"qkT{m}_{s4}") for s4 in range(4)]
                   for m in range(4)]
            # half-swapped duplicates: head data mirrored to the other
            # partition half so consecutive ks scores matmuls can target
            # alternating PE row groups and overlap on hardware
            qkTd = [[qkt_pool.tile([P, 512], BF, tag=f"qkTd{m}_{s4}",
                                   name=f"qkTd{m}_{s4}") for s4 in range(4)]
                    for m in range(4)]
            # V augmented with a ones column, per s-tile [128, head, 66]:
            # [V(64) | 1 | pad] -> PV out at base 0: attn rows 0:64, denom row 64.
            # (matmul PSUM outputs must start at partition 0/64 with <=128/64
            # rows, so odd heads write a temp and DMA into attnT rows 64:128.)
            Vaug = [vaug_pool.tile([P, HG, 66], BF, tag=f"vaug{st}", name=f"vaug{st}")
                    for st in range(ST)]
            attnT = [[attnp.tile([P, 1024], BF, tag=f"attnT{c}_{q2}",
                                 name=f"attnT{c}_{q2}") for q2 in range(2)]
                     for c in range(2)]

            # ---- emission pieces ----
            # The Tile scheduler runs each engine in-order and prioritizes by
            # emission order, so emission is arranged to match the desired
            # execution interleave: exp stream (ACT) is the pacer; projection
            # groups drip into the PE stream between attention ks-pieces.
            qk_rot = [0]

            def qk_proj(s4, m):
                # rotate the contraction order so consecutive groups don't
                # all head-of-line block on the last-arriving xT tile
                rot = qk_rot[0]
                qk_rot[0] = (rot + 1) % ET
                ss = slice(s4 * 512, (s4 + 1) * 512)
                ps = psW.tile([P, 512], F32, tag="ps", name=f"qk{s4}_{m}")
                wt, co = wqk_at[m]
                order = [(rot + i) % ET for i in range(ET)]
                for i, e in enumerate(order):
                    nc.tensor.matmul(
                        ps, lhsT=wt[:, e, co:co + P],
                        rhs=xts[e][:, ss], start=(i == 0), stop=(i == ET - 1))
                nc.vector.tensor_scalar_add(
                    qkT[m][s4], ps, bqk_sb[:, m:m + 1])
                qk_dup(m, s4)

            def qk_dup(m, s4):
                # ACT's HWDGE queue is idle for data traffic; using it keeps
                # these small copies from queueing behind the bulk input DMAs
                nc.scalar.dma_start(
                    qkTd[m][s4][64:128, :], qkT[m][s4][0:64, :])
                nc.scalar.dma_start(
                    qkTd[m][s4][0:64, :], qkT[m][s4][64:128, :])

            def v_proj(st):
                s4, j = st // 4, st % 4
                psv = psW.tile([P, HG * D], F32, tag="ps", name=f"v{st}")
                for e in range(ET):
                    nc.tensor.matmul(
                        psv, lhsT=xts[e][:, st * P:(st + 1) * P],
                        rhs=wv_sb[e], start=(e == 0), stop=(e == ET - 1))
                for h in range(HG):
                    nc.vector.tensor_tensor(
                        Vaug[st][:, h, 0:D],
                        psv[:, h * D:(h + 1) * D],
                        bv_bc[:, h * D:(h + 1) * D], AluOpType.add)
                    nc.vector.memset(Vaug[st][:, h, D:D + 1], 1.0)

            def attn_start(h, q2):
                return [psW.tile([P, 512], F32, tag="ps",
                                 name=f"pv{q2}_{h}_{i}") for i in range(2)]

            def attn_exp_pair(h, q2, kp):
                # ks=2kp uses the natural tiles (this head's partition half),
                # ks=2kp+1 the half-swapped duplicates -> alternating PE row
                # groups, so the interleaved matmuls overlap on hardware.
                pair, hp = h // 2, h % 2
                qm, km = pair, 2 + pair
                bp = hp * 64
                bpd = 64 - bp
                scs = [psS.tile([P, 1024], F32, tag="sc",
                                name=f"sc{q2}_{h}_{2 * kp + i}")
                       for i in range(2)]
                for qh in range(2):
                    for i in range(2):
                        ks = 2 * kp + i
                        ko = (ks % 4) * P
                        if i == 0:
                            lhsT = qkT[km][ks // 4][bp:bp + 64, ko:ko + P]
                            rhs = qkT[qm][q2 * 2 + qh][bp:bp + 64, :]
                        else:
                            lhsT = qkTd[km][ks // 4][bpd:bpd + 64, ko:ko + P]
                            rhs = qkTd[qm][q2 * 2 + qh][bpd:bpd + 64, :]
                        nc.tensor.matmul(
                            scs[i][:, qh * 512:(qh + 1) * 512],
                            lhsT=lhsT, rhs=rhs, start=True, stop=True)
                exs = []
                for i in range(2):
                    ex = expp.tile([P, 1024], BF, tag="ex",
                                   name=f"ex{q2}_{h}_{2 * kp + i}")
                    nc.scalar.activation(ex, scs[i], EXP, scale=0.125)
                    exs.append(ex)
                return exs

            def attn_pv(h, ks, pvs, ex):
                for q in range(2):
                    nc.tensor.matmul(
                        pvs[q][0:65, :],
                        lhsT=Vaug[ks][:, h, 0:65],
                        rhs=ex[:, q * 512:(q + 1) * 512],
                        start=(ks == 0), stop=(ks == ST - 1))

            def attn_ks_stream(h, q2, pvs, filler=None):
                # PV lags one ks-pair behind the exp stream so PSUM-slot
                # waits at head boundaries can't block the scores/exp chain
                exs = []
                for kp in range(ST // 2):
                    exs.extend(attn_exp_pair(h, q2, kp))
                    if filler:
                        filler(2 * kp)
                    if kp >= 1:
                        attn_pv(h, 2 * kp - 2, pvs, exs[2 * kp - 2])
                        attn_pv(h, 2 * kp - 1, pvs, exs[2 * kp - 1])
                attn_pv(h, ST - 2, pvs, exs[ST - 2])
                attn_pv(h, ST - 1, pvs, exs[ST - 1])

            def attn_norm(h, q2, pvs):
                pair, hp = h // 2, h % 2
                even = hp == 0
                for q in range(2):
                    qi = q * 512
                    # evacuate attn+denom rows to SBUF right away so the
                    # PSUM accumulator frees for the next head
                    pvc = rbp.tile([P, 512], F32R, tag="pvc")
                    nc.vector.tensor_copy(pvc[0:65, :], pvs[q][0:65, :])
                    # broadcast the denom row across partitions with a K=1
                    # outer product (ones x denom row) on PE, written into
                    # the dying PV accumulator's attn rows (already copied
                    # out to pvc) - costs no extra PSUM and no DMA
                    nc.tensor.matmul(
                        pvs[q][0:64, :], lhsT=ones_t[64:65, 0:64],
                        rhs=pvc[64:65, :],
                        start=True, stop=True)
                    rb = rbp.tile([P, 512], F32, tag="rb")
                    nc.vector.reciprocal_approx_fast(
                        rb[0:64, :], pvs[q][0:64, :])
                    if even:
                        nc.vector.tensor_tensor(
                            attnT[pair][q2][0:64, qi:qi + 512],
                            pvc[0:64, :], rb[0:64, :], AluOpType.mult)
                    else:
                        tmp = rbp.tile([64, 512], BF, tag="atmp")
                        nc.vector.tensor_tensor(
                            tmp, pvc[0:64, :], rb[0:64, :], AluOpType.mult)
                        nc.gpsimd.dma_start(
                            attnT[pair][q2][64:128, qi:qi + 512], tmp)

            def out_proj_st(q2, st):
                # q2=0 runs concurrently with attention(q2=1) -> psW slots.
                # q2=1 is the tail: the scores pool is idle -> use its 2-bank
                # slots as [128,1024] tiles; accumulate pair1 first (head
                # order makes pair0 last ready).
                corder = (0, 1) if q2 == 0 else (1, 0)
                so = (st % 8) * P
                if q2 == 0:
                    pos = [psW.tile([P, 512], F32, tag="ps",
                                    name=f"po{st}_{e2}") for e2 in range(2)]
                else:
                    pow_ = psS.tile([P, 1024], F32, tag="sc", name=f"po{st}")
                    pos = [pow_[:, 0:512], pow_[:, 512:1024]]
                for i, c in enumerate(corder):
                    for e2 in range(2):
                        nc.tensor.matmul(
                            pos[e2],
                            lhsT=attnT[c][q2][:, so:so + P],
                            rhs=wout_sb[c][:, e2 * 512:(e2 + 1) * 512],
                            start=(i == 0), stop=(i == 1))
                if q2 == 0:
                    for e2 in range(2):
                        ob = outsb.tile([P, 512], F32, tag="ob")
                        nc.vector.tensor_tensor(
                            ob, pos[e2], bout_bc[:, e2 * 512:(e2 + 1) * 512],
                            AluOpType.add)
                        (nc.sync if st % 2 else nc.gpsimd).dma_start(
                            out_d[st * P:(st + 1) * P,
                                  e2 * 512:(e2 + 1) * 512], ob)
                else:
                    ob = outsb.tile([P, 1024], F32, tag="ob2")
                    if st % 2 == 0:
                        # ACT is idle in the tail: copy there, bias on DVE
                        nc.scalar.activation(
                            ob, pow_, mybir.ActivationFunctionType.Copy)
                        nc.vector.tensor_tensor(
                            ob, ob, bout_bc, AluOpType.add)
                    else:
                        nc.vector.tensor_tensor(
                            ob, pow_, bout_bc, AluOpType.add)
                    (nc.sync if st % 2 else nc.gpsimd).dma_start(
                        out_d[st * P:(st + 1) * P, :], ob)

            # ---- input loads ----
            xts = []
            for e in range(ET):
                t = xin.tile([P, S], BF, tag="xt", name=f"xt{e}")
                qs[e % 2].dma_start(t, xT_d[e * P:(e + 1) * P, :])
                xts.append(t)
            wv_all = consts.tile([P, ET, HG * D], BF, tag="wv", name="wv_all")
            nc.sync.dma_start(
                wv_all, wv_d.rearrange("(e p) c -> p e c", p=P))
            wv_sb = [wv_all[:, e, :] for e in range(ET)]
            wqk13 = consts.tile([P, ET, 2 * P], BF, tag="wqk13", name="wqk13")
            nc.gpsimd.dma_start(
                wqk13, wqk13_d.rearrange("(e p) c -> p e c", p=P))
            # m-tile -> (sbuf tile, column offset): 0,2 in wqk02; 1,3 in wqk13
            wqk_at = {0: (wqk02, 0), 2: (wqk02, P), 1: (wqk13, 0),
                      3: (wqk13, P)}
            wout_all = consts.tile([P, 2, E], BF, tag="wout", name="wout_all")
            nc.scalar.dma_start(
                wout_all, wout_d.rearrange("(c p) n -> p c n", p=P))
            wout_sb = [wout_all[:, c, :] for c in range(2)]
            bqk_sb = consts.tile([P, 4], F32, tag="bqk")
            nc.sync.dma_start(bqk_sb, bqk_d)
            bv_bc = consts.tile([P, HG * D], F32, tag="bv")
            nc.scalar.dma_start(bv_bc, bv_d.to_broadcast([P, HG * D]))
            bout_bc = consts.tile([P, E], F32, tag="bout")
            nc.gpsimd.dma_start(bout_bc, bout_d.to_broadcast([P, E]))
            ones_f = consts.tile([P, 64], F32, tag="onesf")
            nc.vector.memset(ones_f, 1.0)
            ones_t = consts.tile([P, 64], F32R, tag="ones")
            nc.vector.tensor_copy(ones_t, ones_f)

            # ---- schedule ----
            # h0's minimal prerequisites, accumulated e-major and interleaved
            # across three PSUM banks so the whole block completes right
            # after the last xT tile arrives (each group owns its bank;
            # the in-order PE stream stays xT-arrival paced).
            pre = [(0, 0), (1, 0), (0, 2)]   # (s4, m)
            pre_ps = {}
            for s4, m in pre:
                pre_ps[(s4, m)] = psW.tile(
                    [P, 512], F32, tag="ps", name=f"qk{s4}_{m}")
            for e in range(ET):
                for s4, m in pre:
                    wt, co = wqk_at[m]
                    nc.tensor.matmul(
                        pre_ps[(s4, m)],
                        lhsT=wt[:, e, co:co + P],
                        rhs=xts[e][:, s4 * 512:(s4 + 1) * 512],
                        start=(e == 0), stop=(e == ET - 1))
            for s4, m in pre:
                nc.vector.tensor_scalar_add(
                    qkT[m][s4], pre_ps[(s4, m)], bqk_sb[:, m:m + 1])
            for s4, m in pre:
                qk_dup(m, s4)
            # h0 q2=0: v-projection dripped just-in-time for PV, and the
            # remaining k-pair0 groups dripped just ahead of their ks range
            def h0_filler(ks):
                v_proj(ks)
                v_proj(ks + 1)
                if ks in (0, 4, 8):
                    qk_proj(ks // 4 + 1, 2)
            pvs = attn_start(0, 0)
            attn_ks_stream(0, 0, pvs, h0_filler)
            attn_norm(0, 0, pvs)
            # h1 q2=0; drip pass-B projections through the stream
            fillers = [(1, 0), (1, 1), (3, 0), (3, 1), (3, 2), (3, 3),
                       (0, 2), (0, 3), (1, 2), (1, 3)]  # (m, s4)
            def h1_filler(ks):
                n = 2 if ks < 4 else 1
                for _ in range(n):
                    if fillers:
                        m, s4 = fillers.pop(0)
                        qk_proj(s4, m)
            pvs = attn_start(1, 0)
            attn_ks_stream(1, 0, pvs, h1_filler)
            attn_norm(1, 0, pvs)
            for h in (2, 3):
                pvs = attn_start(h, 0)
                attn_ks_stream(h, 0, pvs)
                attn_norm(h, 0, pvs)
            # q2=1 with q2=0's out-projection spread over h2+h3 streams
            opq = list(range(8))
            for hi, h in enumerate((2, 3, 1, 0)):
                def op_filler(ks, hi=hi):
                    if hi < 2 and ks % 4 == 2 and opq:
                        out_proj_st(0, opq.pop(0))
                pvs = attn_start(h, 1)
                attn_ks_stream(h, 1, pvs, op_filler)
                attn_norm(h, 1, pvs)
            for st in range(8, 16):
                out_proj_st(1, st)

    nc.compile()
    return nc


def get_program():
    global _COMPILED
    if _COMPILED is None:
        _COMPILED = build_program()
    return _COMPILED


def make_in_maps(x, W_qkv, b_qkv, W_out, b_out):
    """Host-side shard/permute/cast. Returns list of per-core input dicts."""
    x = np.asarray(x, dtype=np.float32)
    W_qkv = np.asarray(W_qkv, dtype=np.float32)
    b_qkv = np.asarray(b_qkv, dtype=np.float32)
    W_out = np.asarray(W_out, dtype=np.float32)
    b_out = np.asarray(b_out, dtype=np.float32)

    in_maps = []
    for c in range(N_CORES):
        b = c // 4
        g = c % 4
        heads = [4 * g + i for i in range(HG)]
        xT = np.ascontiguousarray(x[b].T).astype(BF16)
        wqk = np.empty((E, 4 * P), np.float32)
        bqk_flat = np.empty((4 * P,), np.float32)
        wv = np.empty((E, HG * D), np.float32)
        bv = np.empty((1, HG * D), np.float32)
        wout = np.empty((HG * D, E), np.float32)
        for i, h in enumerate(heads):
            base = h * 3 * D
            wqk[:, i * D:(i + 1) * D] = W_qkv[:, base:base + D]
            wqk[:, 256 + i * D:256 + (i + 1) * D] = W_qkv[:, base + D:base + 2 * D]
            bqk_flat[i * D:(i + 1) * D] = b_qkv[base:base + D]
            bqk_flat[256 + i * D:256 + (i + 1) * D] = b_qkv[base + D:base + 2 * D]
            wv[:, i * D:(i + 1) * D] = W_qkv[:, base + 2 * D:base + 3 * D]
            bv[0, i * D:(i + 1) * D] = b_qkv[base + 2 * D:base + 3 * D]
            wout[i * D:(i + 1) * D, :] = W_out[h * D:(h + 1) * D, :]
        bqk = np.ascontiguousarray(bqk_flat.reshape(4, P).T)  # [128, 4]
        wqk02 = np.concatenate(
            [wqk[:, 0:P], wqk[:, 2 * P:3 * P]], axis=1)
        wqk13 = np.concatenate(
            [wqk[:, P:2 * P], wqk[:, 3 * P:4 * P]], axis=1)
        in_maps.append({
            "xT": xT,
            "wqk02": wqk02.astype(BF16),
            "wqk13": wqk13.astype(BF16),
            "wv": wv.astype(BF16),
            "wout": wout.astype(BF16),
            "bqk": bqk,
            "bv": bv,
            "bout": (b_out / 4.0).reshape(1, E),
        })
    return in_maps


def gather_outputs(results):
    """Sum the 4 head-group partials per batch."""
    out = np.zeros((B, S, E), np.float32)
    for c in range(N_CORES):
        out[c // 4] += results[c]["out"]
    return out


def run(in_maps, trace=False, **kwargs):
    nc = get_program()
    return run_bass_kernel_spmd(nc, in_maps, list(range(N_CORES)),
                                trace=trace, **kwargs)


def kernel(x, W_qkv, b_qkv, W_out, b_out):
    in_maps = make_in_maps(x, W_qkv, b_qkv, W_out, b_out)
    res = run(in_maps)
    return gather_outputs(res.results)
